# revision 37
# baseline (speedup 1.0000x reference)
"""Attention-pooling Trainium2 kernel (8-core SPMD), v13.

Math (matches the jax reference):
    x   = tanh(H @ w1.T); s = x @ w2.T
    S   = segment_softmax(s, batch)   (plain exp - |s|<4, no max-sub)
    out = segment_sum(S * H)

v10 over v9:
  - BOTH H copies ship as float8_e3m4 at 2x scale (score ht8 as before,
    and now the accumulation copy hn8 too): 41.1 MB/core total DMA
    (was 57.8). Host divides the numerator by 2 at assembly. Simulated
    rel err 1.60e-2 (gate 2e-2).
  - e16o exported as f16 (ebuf is f16 end to end).
  - software-pipelined emission: ACT runs tanh back-to-back; each
    chunk's exp/s-matmuls lag one chunk and its accumulation lags two,
    so the strict-FIFO ACT queue never stalls behind PE work (v9 lost
    ~1us/chunk there). PE queue order per step c: score(c), s(c-1),
    accum(c-2). All input DMAs issue on the sync HWDGE ring, keeping
    the ACT queue free of DMA triggers.

v13 over v10 (bass asserts matmul psum output must be f32, so the
f16-psum ideas are out):
  - consts preload via sync HWDGE, w1t/ht(0)/hn(0) first so the first
    score matmul fires earlier.
  - e16o exported in slices every 8 chunks (shorter tail).
  - exp batched per chunk-PAIR ([D,64] psum tile) for chunks 0..27,
    with pair-batched weight construction too (fewer ACT/DVE ops);
    the last three chunks run as singles so the drain keeps the v10
    lag-2 shape (v12's uniform pairing cost +2us of tail).
  - tail-only output DMAs ride the sync HWDGE ring (no input loads
    left to block; ~1.4us less completion latency each).
"""

import os
import numpy as np
import ml_dtypes

D = 128
N_CORES = 8
K = 8              # max segment span per block
CBLK = 32          # blocks per chunk (4096 node slots)
F16 = np.float16
F8 = ml_dtypes.float8_e3m4


# ----------------------------------------------------------------- host prep

def _shard_cuts(batch, n_cores):
    n = batch.shape[0]
    cuts = [0]
    for k in range(1, n_cores):
        t = n * k // n_cores
        cuts.append(int(np.searchsorted(batch, batch[t], side="left")))
    cuts.append(n)
    return cuts


def _greedy_blocks(batch, lo, hi, k_span):
    starts, counts, bases = [], [], []
    i = lo
    while i < hi:
        base = int(batch[i])
        jmax = min(i + 128, hi)
        j = int(np.searchsorted(batch[i:jmax], base + k_span, side="left")) + i
        starts.append(i)
        counts.append(j - i)
        bases.append(base)
        i = j
    return np.array(starts), np.array(counts), np.array(bases)


def _prep_core(H, batch, lo, hi, nblk):
    starts, counts, bases = _greedy_blocks(batch, lo, hi, K)
    nb = len(starts)
    assert nb <= nblk
    nslot = nblk * 128
    slot_node = np.full(nslot, -1, dtype=np.int64)
    for b in range(nb):
        s, c = starts[b], counts[b]
        slot_node[b * 128:b * 128 + c] = np.arange(s, s + c)
    valid = slot_node >= 0

    Hp = np.zeros((nslot, D), dtype=np.float32)
    Hp[valid] = H[slot_node[valid]]
    h2 = np.clip(Hp * 2.0, -15.5, 15.5)
    ht8 = np.ascontiguousarray(h2.T).astype(F8)              # [128, nslot]
    hn8 = np.ascontiguousarray(
        h2.astype(F8).reshape(nblk // CBLK, CBLK, 128, D)
        .transpose(0, 2, 1, 3))                              # [nc,128,32,128]

    brel = np.full(nslot, -1.0, dtype=np.float32)
    brel[valid] = (batch[slot_node[valid]]
                   - np.repeat(bases, 128)[: nb * 128][valid[: nb * 128]]
                   ).astype(np.float32)
    brel = np.ascontiguousarray(brel.reshape(nblk, 128).T).astype(F16)

    base_full = np.full(nblk, -1, dtype=np.int64)
    base_full[:nb] = bases
    return dict(ht8=ht8, hn8=hn8, brel=brel, bases=base_full,
                slot_node=slot_node)


# ------------------------------------------------------------- device kernel

def _build_program(nblk):
    import concourse.bacc as bacc
    import concourse.tile as tile
    from concourse import mybir

    f8 = mybir.dt.float8e3
    f16 = mybir.dt.float16
    f32 = mybir.dt.float32
    nchunk = nblk // CBLK
    CS = CBLK * 128

    nc = bacc.Bacc("TRN2", target_bir_lowering=False, debug=False,
                   num_devices=N_CORES)
    ht_d = nc.dram_tensor("ht8", [D, nblk * 128], f8, kind="ExternalInput")
    hn_d = nc.dram_tensor("hn8", [nchunk, D, CBLK, D], f8,
                          kind="ExternalInput")
    brel_d = nc.dram_tensor("brel", [D, nblk], f16, kind="ExternalInput")
    iota_d = nc.dram_tensor("iota", [D, 2 * CBLK, K], f16,
                            kind="ExternalInput")
    w1_d = nc.dram_tensor("w1s", [D, D], f16, kind="ExternalInput")
    w2_d = nc.dram_tensor("w2t", [D, 1], f16, kind="ExternalInput")
    num_d = nc.dram_tensor("numout", [nchunk * 2, D, 512], f16,
                           kind="ExternalOutput")
    e_d = nc.dram_tensor("e16o", [D, nblk], f16, kind="ExternalOutput")

    with tile.TileContext(nc) as tc:
        with tc.tile_pool(name="const", bufs=1) as constp, \
             tc.tile_pool(name="ht", bufs=8) as htp, \
             tc.tile_pool(name="hn", bufs=8) as hnp, \
             tc.tile_pool(name="xt", bufs=3) as xtp, \
             tc.tile_pool(name="wm", bufs=6) as wmp, \
             tc.tile_pool(name="nex", bufs=4) as nexp, \
             tc.tile_pool(name="px", bufs=2, space="PSUM") as pxp, \
             tc.tile_pool(name="ps", bufs=2, space="PSUM") as psp, \
             tc.tile_pool(name="pw", bufs=2, space="PSUM") as pwp:

            w1t = constp.tile([D, D], f16)
            nc.sync.dma_start(w1t[:], w1_d.ap())
            w2t = constp.tile([D, 1], f16)
            iotag = constp.tile([D, 2 * CBLK, K], f16)
            brel = constp.tile([D, nblk], f16)
            ebuf = constp.tile([D, nblk], f16)

            hns = {}
            xts = {}
            wms = {}      # even chunk e -> [D, 2*CBLK, K] tile for (e, e+1)
            psq = {}      # even chunk e -> [D, 2*CBLK] psum tile
            eexp = 0      # e16o export watermark (chunks)

            npair = max(0, ((nchunk - 3) // 2) * 2)  # chunks < npair pair up

            def emit_wm(e, hi):
                # one-hot * e weights for chunks [e, e+hi)
                w = CBLK * hi
                br_b = brel[:, e * CBLK:e * CBLK + w] \
                    .unsqueeze(2).broadcast_to([D, w, K])
                ev_b = ebuf[:, e * CBLK:e * CBLK + w] \
                    .unsqueeze(2).broadcast_to([D, w, K])
                wt = wmp.tile([D, 2 * CBLK, K], f16, name="wt")
                wm = wmp.tile([D, 2 * CBLK, K], f16, name="wm")
                nc.vector.tensor_tensor(wt[:, :w], iotag[:, :w], br_b,
                                        mybir.AluOpType.is_equal)
                nc.vector.tensor_tensor(wm[:, :w], wt[:, :w], ev_b,
                                        mybir.AluOpType.mult)
                wms[e] = wm

            def emit_s(c):
                # s-matmuls (xt stationary, w2 moving: big operand rides
                # the LDW port); chunks < npair share a pair psum tile and
                # exp on the odd member; later chunks exp singly
                xt = xts.pop(c)
                if c < npair:
                    e = c - (c % 2)
                    if c % 2 == 0:
                        psq[e] = psp.tile([D, 2 * CBLK], f32,
                                          name="ps_pair")
                    ps = psq[e]
                    off = (c % 2) * CBLK
                else:
                    e = c
                    psq[e] = psp.tile([D, 2 * CBLK], f32, name="ps_pair")
                    ps = psq[e]
                    off = 0
                for b in range(CBLK):
                    nc.tensor.matmul(ps[:, off + b:off + b + 1],
                                     xt[:, b * 128:(b + 1) * 128],
                                     w2t[:], start=True, stop=True)
                if c >= npair:
                    nc.scalar.activation(ebuf[:, c * CBLK:(c + 1) * CBLK],
                                         psq.pop(e)[:, :CBLK],
                                         mybir.ActivationFunctionType.Exp)
                    emit_wm(c, 1)
                elif c % 2 == 1:
                    nc.scalar.activation(ebuf[:, e * CBLK:(e + 2) * CBLK],
                                         psq.pop(e)[:],
                                         mybir.ActivationFunctionType.Exp)
                    emit_wm(e, 2)

            def emit_accum(c, tail=False):
                # packed accumulation: 4 blocks per matmul, diag valid
                if c < npair:
                    e = c - (c % 2)
                    woff = (c % 2) * CBLK
                    wm = wms[e]
                    if c % 2 == 1:
                        del wms[e]
                else:
                    woff = 0
                    wm = wms.pop(c)
                hn = hns.pop(c)
                for h in range(2):
                    pw = pwp.tile([D, 512], f32)
                    for g in range(4):
                        t0 = woff + h * 16 + 4 * g
                        nc.tensor.matmul(
                            pw[32 * g:32 * (g + 1), :],
                            wm[:, t0:t0 + 4, :],
                            hn[:, t0 - woff:t0 - woff + 4, :],
                            start=True, stop=True,
                            tile_position=(0, 32 * g),
                            skip_group_check=True)
                    nex = nexp.tile([D, 512], f16)
                    nc.vector.tensor_copy(nex[:], pw[:])
                    eng = nc.sync if tail else nc.gpsimd
                    eng.dma_start(num_d.ap()[2 * c + h], nex[:])

            for c in range(nchunk):
                ht = htp.tile([D, CS], f8)
                nc.sync.dma_start(ht[:], ht_d.ap()[:, c * CS:(c + 1) * CS])
                hn = hnp.tile([D, CBLK, D], f8)
                nc.sync.dma_start(hn[:], hn_d.ap()[c])
                hns[c] = hn
                if c == 0:
                    # small consts after the first data loads on the ring
                    nc.sync.dma_start(w2t[:], w2_d.ap())
                    nc.sync.dma_start(iotag[:], iota_d.ap())
                    nc.sync.dma_start(brel[:], brel_d.ap())

                last = c == nchunk - 1
                if last:
                    # last chunk: 4 separate xt tiles so the tail's
                    # s/exp/wm/accum can run per 16-block half while
                    # the final tanh ops are still streaming
                    xt4 = [constp.tile([D, 1024], f16, name=f"xtL{j}")
                           for j in range(4)]
                else:
                    xt = xtp.tile([D, CS], f16)
                    xts[c] = xt
                for j in range(CBLK // 8):
                    px = pxp.tile([D, 1024], f32)
                    for jj in range(2):
                        nc.tensor.matmul(px[:, jj * 512:(jj + 1) * 512],
                                         w1t[:],
                                         ht[:, (2 * j + jj) * 512:(2 * j + jj + 1) * 512],
                                         start=True, stop=True)
                    dst = xt4[j][:] if last \
                        else xt[:, j * 1024:(j + 1) * 1024]
                    nc.scalar.activation(dst, px[:],
                                         mybir.ActivationFunctionType.Tanh)

                if c >= 1:
                    emit_s(c - 1)
                if c >= 3 and c - 3 < npair:
                    emit_accum(c - 3)
                if c >= 2 and c - 2 >= npair:
                    emit_accum(c - 2)
                if last:
                    # first half of the last chunk: s-matmuls wait only
                    # on tanh j=0,1; exp+weights for blocks 0..15.
                    # Emitted after this step's other PE work so nothing
                    # queues behind the tanh wait.
                    psL = psp.tile([D, 2 * CBLK], f32, name="ps_pair")
                    for b in range(16):
                        nc.tensor.matmul(psL[:, b:b + 1],
                                         xt4[b // 8][:, (b % 8) * 128:(b % 8 + 1) * 128],
                                         w2t[:], start=True, stop=True)
                    nc.scalar.activation(
                        ebuf[:, c * CBLK:c * CBLK + 16], psL[:, :16],
                        mybir.ActivationFunctionType.Exp)
                    brA = brel[:, c * CBLK:c * CBLK + 16] \
                        .unsqueeze(2).broadcast_to([D, 16, K])
                    evA = ebuf[:, c * CBLK:c * CBLK + 16] \
                        .unsqueeze(2).broadcast_to([D, 16, K])
                    wtA = wmp.tile([D, 16, K], f16, name="wtA")
                    wmA = wmp.tile([D, 16, K], f16, name="wmA")
                    nc.vector.tensor_tensor(wtA[:], iotag[:, :16], brA,
                                            mybir.AluOpType.is_equal)
                    nc.vector.tensor_tensor(wmA[:], wtA[:], evA,
                                            mybir.AluOpType.mult)
                if c > 0 and c % 8 == 0:
                    # export finished e slices early to shorten the tail
                    nc.gpsimd.dma_start(
                        e_d.ap()[:, eexp * CBLK:(c - 2) * CBLK],
                        ebuf[:, eexp * CBLK:(c - 2) * CBLK])
                    eexp = c - 2

            L = nchunk - 1
            # accum(L-1) first: it has no last-chunk deps and overlaps
            # the final tanh ops
            emit_accum(L - 1, tail=True)
            # second half of the last chunk
            for b in range(16, CBLK):
                nc.tensor.matmul(psL[:, b:b + 1],
                                 xt4[b // 8][:, (b % 8) * 128:(b % 8 + 1) * 128],
                                 w2t[:], start=True, stop=True)
            nc.scalar.activation(ebuf[:, L * CBLK + 16:(L + 1) * CBLK],
                                 psL[:, 16:32],
                                 mybir.ActivationFunctionType.Exp)
            brB = brel[:, L * CBLK + 16:(L + 1) * CBLK] \
                .unsqueeze(2).broadcast_to([D, 16, K])
            evB = ebuf[:, L * CBLK + 16:(L + 1) * CBLK] \
                .unsqueeze(2).broadcast_to([D, 16, K])
            wtB = wmp.tile([D, 16, K], f16, name="wtB")
            wmB = wmp.tile([D, 16, K], f16, name="wmB")
            nc.vector.tensor_tensor(wtB[:], iotag[:, :16], brB,
                                    mybir.AluOpType.is_equal)
            nc.vector.tensor_tensor(wmB[:], wtB[:], evB,
                                    mybir.AluOpType.mult)
            # accumulation for the last chunk, per half
            hnL = hns.pop(L)
            for h, wmX in ((0, wmA), (1, wmB)):
                pw = pwp.tile([D, 512], f32)
                for g in range(4):
                    nc.tensor.matmul(
                        pw[32 * g:32 * (g + 1), :],
                        wmX[:, 4 * g:4 * g + 4, :],
                        hnL[:, h * 16 + 4 * g:h * 16 + 4 * g + 4, :],
                        start=True, stop=True,
                        tile_position=(0, 32 * g),
                        skip_group_check=True)
                nex = nexp.tile([D, 512], f16)
                nc.vector.tensor_copy(nex[:], pw[:])
                nc.sync.dma_start(num_d.ap()[2 * L + h], nex[:])

            nc.sync.dma_start(e_d.ap()[:, eexp * CBLK:],
                              ebuf[:, eexp * CBLK:])

    nc.compile()
    return nc


# ------------------------------------------------------------------ assembly

def _assemble(size, cores, results):
    num = np.zeros((size, D), dtype=np.float32)
    den = np.zeros(size, dtype=np.float32)
    for core, res in zip(cores, results):
        bases = core["bases"]
        nblk = bases.shape[0]
        # numerator: [ntile, 128, 512]; block t = tile*16 + 4g + i valid at
        # rows 32g+8i+k, cols 128i+f; values are sum(e * 2H) -> halved below
        no = np.asarray(res["numout"], dtype=np.float32)
        ntile = no.shape[0]
        no = no.reshape(ntile, 4, 4, K, 4, D)     # [tile, g, i, k, b, f]
        i4 = np.arange(4)
        vals = no[:, :, i4, :, i4, :]             # [i, tile, g, k, f]
        vals = np.moveaxis(vals, 0, 2)            # [tile, g, i, k, f]
        vals = np.ascontiguousarray(vals).reshape(nblk * K, D)
        colseg = (np.repeat(bases, K) +
                  np.tile(np.arange(K), nblk))
        ok = np.repeat(bases >= 0, K) & (colseg < size) & (colseg >= 0)
        np.add.at(num, colseg[ok], vals[ok])
        # denominator from exported device e (f16 = device weights)
        e = np.ascontiguousarray(res["e16o"].T).reshape(nblk * 128)
        e = e.astype(np.float16).astype(np.float32)
        sn = core["slot_node"]
        valid = sn >= 0
        np.add.at(den, core["batch_slot"][valid], e[valid])
    return (0.5 * num) / (den + 1e-16)[:, None]


# -------------------------------------------------------------------- kernel

def kernel(H, batch, w1, w2, size):
    H = np.asarray(H, dtype=np.float32)
    batch = np.asarray(batch).astype(np.int64)
    w1 = np.asarray(w1, dtype=np.float32)
    w2 = np.asarray(w2, dtype=np.float32)
    size = int(size)
    n = H.shape[0]
    assert H.shape[1] == D

    cuts = _shard_cuts(batch, N_CORES)
    nb_max = 0
    for c in range(N_CORES):
        starts, _, _ = _greedy_blocks(batch, cuts[c], cuts[c + 1], K)
        nb_max = max(nb_max, len(starts))
    nblk = ((nb_max + CBLK - 1) // CBLK) * CBLK

    cores = []
    in_maps = []
    iota = np.broadcast_to(np.arange(K, dtype=F16), (D, 2 * CBLK, K)).copy()
    w1s = np.ascontiguousarray(w1.T * 0.5).astype(F16)
    w2t = np.ascontiguousarray(w2.reshape(1, D).T).astype(F16)
    for c in range(N_CORES):
        lo, hi = cuts[c], cuts[c + 1]
        core = _prep_core(H, batch, lo, hi, nblk)
        sn = core["slot_node"]
        core["batch_slot"] = np.where(sn >= 0, batch[np.clip(sn, 0, n - 1)], 0)
        cores.append(core)
        in_maps.append({
            "ht8": core["ht8"], "hn8": core["hn8"], "brel": core["brel"],
            "iota": iota, "w1s": w1s, "w2t": w2t,
        })

    nc = _build_program(nblk)

    from concourse.bass_utils import run_bass_kernel_spmd
    trace = bool(os.environ.get("ATTN_TRACE"))
    kwargs = {}
    if trace:
        import sys, types
        import antenv
        if "antenv.axon_hooks" not in sys.modules:
            mod = types.ModuleType("antenv.axon_hooks")
            _h = {}
            mod.set_axon_ntff_profile_hook = lambda h: _h.__setitem__("h", h)
            mod.get_axon_ntff_profile_hook = lambda: _h.get("h")
            sys.modules["antenv.axon_hooks"] = mod
            antenv.axon_hooks = mod
        from trn_agent_boot.trn_boot import _ntff_profile_via_ctypes
        sys.modules["antenv.axon_hooks"].set_axon_ntff_profile_hook(
            _ntff_profile_via_ctypes("/opt/axon/libaxon_pjrt.so"))
        from concourse import bass_utils as _bu
        _bu.upload_artifacts = lambda tmpdir: f"local://{tmpdir}"
        tmpdir = os.environ.get("ATTN_TRACE_DIR") or None
        kwargs = dict(trace=True, tmpdir=tmpdir)

    res = run_bass_kernel_spmd(nc, in_maps, list(range(N_CORES)), **kwargs)
    kernel.last_exec_time_ns = res.exec_time_ns
    out = _assemble(size, cores, [res.results[c] for c in range(N_CORES)])
    return out
